# revision 19
# baseline (speedup 1.0000x reference)
"""Self-contained Trainium2 Bass kernel for nn_AttentionModel (B=4, S=2048, E=1024).

Model: q/k/v linear projections + scaled-dot-product attention (scale = sqrt(E)).

Sharding (8 NeuronCores): core c handles batch b=c//2, query-row half h=c%2
(1024 q rows). k/v projections are split across the core pair (each projects
its own 1024 k-rows) and exchanged with a pair-local AllGather
(replica_groups [[0,1],[2,3],[4,5],[6,7]]); both halves are read back from
the collective output in absolute rank order, so the program stays fully
SPMD-uniform with no core-dependent addressing.

Device algorithm per core (all matmuls bf16 with fp32 PSUM accumulation):
  qT_proj[f,q]  = WqT.T @ qT_in   (+bq via ACT bias on eviction)
  kT_proj[f,k]  = WkT.T @ kT_in   (+bk)   [own half, then pair AllGather]
  v_proj [k,f]  = vT_in.T @ WvT   [own half, then pair AllGather;
                                   bias bv applied on host after gather]
  scoresT[k,q]  = kT_proj.T @ qT_proj          (per 128k x 512q psum tile)
  expT   [k,q]  = exp(scoresT / sqrt(E))       (ACT, no max-subtraction:
                                                logits are O(+-6), fp32-safe)
  out_un [q,f]  = expT.T @ v_proj              (accumulate over k chunks)
  sums   [q,1]  = expT.T @ ones                (same stationary operand)
  out    [q,f]  = out_un * (1/sums)            (per-partition ACT scale)

Host pre-tiles every input into the [128, outer, free] SBUF layout in bf16, so
the device performs no transposes or casts on the inputs.
"""

import sys

for _p in ("/opt/trn_rl_repo", "/root/.axon_site/_ro/trn_rl_repo"):
    if _p not in sys.path:
        sys.path.insert(0, _p)

import numpy as np
import ml_dtypes

import concourse.bacc as bacc
import concourse.mybir as mybir
import concourse.tile as tile
from concourse.bass_utils import run_bass_kernel_spmd

B, S, E = 4, 2048, 1024
P = 128
SQ = S // 2          # q rows per core
N_CORES = 8
EO = E // P          # 8  e-outer chunks
FO = E // P          # 8  f-outer chunks
KC = S // P          # 16 k-row chunks
QB = SQ // 512       # 2  q 512-blocks
INV_SCALE = float(1.0 / np.sqrt(np.float32(E)))

BF16 = mybir.dt.bfloat16
F32 = mybir.dt.float32

_BUILD_CACHE: dict = {}


def _build(with_mask: bool):
    nc = bacc.Bacc(
        "TRN2",
        target_bir_lowering=False,
        debug=False,
        enable_asserts=False,
        num_devices=N_CORES,
    )

    # qt/kt/wqt/wkt arrive as fo/qb/kb slabs so the first matmul chain only
    # needs ~0.75MB of DMA before PE can start.
    qt_d = nc.declare_dram_parameter("qt", [QB, P, EO, 512], BF16, isOutput=False)
    kt_d = nc.declare_dram_parameter("kt", [2, P, EO, 512], BF16, isOutput=False)
    vt_d = nc.declare_dram_parameter("vt", [KC // 2, P, EO, P], BF16, isOutput=False)
    wqt_d = nc.declare_dram_parameter("wqt", [FO, P, EO, P], BF16, isOutput=False)
    wkt_d = nc.declare_dram_parameter("wkt", [FO, P, EO, P], BF16, isOutput=False)
    wvt_d = nc.declare_dram_parameter("wvt", [2, P, EO, 512], BF16, isOutput=False)
    bq_d = nc.declare_dram_parameter("bq", [P, FO], F32, isOutput=False)
    bk_d = nc.declare_dram_parameter("bk", [P, FO], F32, isOutput=False)
    if with_mask:
        mask_d = nc.declare_dram_parameter("maskt", [P, KC, SQ], BF16, isOutput=False)
    out_d = nc.declare_dram_parameter("out", [P, SQ // P, E], F32, isOutput=True)

    # pair-AllGather staging buffers (internal DRAM)
    # NOTE: Shared-output collectives need >4-core groups; for 2-core pair
    # groups the output must stay a Local internal tensor. The k exchange is
    # split into two kb-halves so the first AllGather launches ~15us earlier
    # and scores can start on its chunks while the rest is in flight.
    cck_in = [nc.dram_tensor(f"cck_in{kb}", [P, FO, 512], BF16) for kb in range(2)]
    cck_out = [nc.dram_tensor(f"cck_out{kb}", [2, P, FO, 512], BF16)
               for kb in range(2)]
    ccv_in = nc.dram_tensor("ccv_in", [P, KC // 2, E], BF16)
    ccv_out = nc.dram_tensor("ccv_out", [2, P, KC // 2, E], BF16)
    REPLICA_GROUPS = [[0, 1], [2, 3], [4, 5], [6, 7]]

    with tile.TileContext(nc) as tc:
        with (
            tc.tile_pool(name="const", bufs=1) as const,
            tc.tile_pool(name="proj", bufs=1) as proj,
            tc.tile_pool(name="ppsum", bufs=2, space="PSUM") as ppsum,
            tc.tile_pool(name="opsum_a", bufs=2, space="PSUM") as opsum_a,
            tc.tile_pool(name="opsum_b", bufs=2, space="PSUM") as opsum_b,
            tc.tile_pool(name="spsum", bufs=2, space="PSUM") as spsum,
        ):
            ones_sb = const.tile([P, 1], BF16)
            nc.any.memset(ones_sb[:], 1.0)
            bq_sb = const.tile([P, FO], F32)
            nc.sync.dma_start(out=bq_sb[:], in_=bq_d[:])
            bk_sb = const.tile([P, FO], F32)
            nc.sync.dma_start(out=bk_sb[:], in_=bk_d[:])

            # persistent projected tensors
            qt_proj = proj.tile([P, FO, SQ], BF16)   # [f-inner, fo, q]
            kt_proj = proj.tile([P, FO, S], BF16)    # [f-inner, fo, k]
            v_proj = proj.tile([P, KC, E], BF16)     # [k-inner, kc, f]

            # ---------------- phase A: projections ----------------
            with tc.tile_pool(name="io", bufs=1) as io:
                # Priming DMAs in consumption order: k-proj inputs first (its
                # AllGathers must launch earliest), then v, then q.
                wk_sb = io.tile([P, EO, E], BF16)
                kt_in = io.tile([P, EO, SQ], BF16)
                nc.sync.dma_start(out=wk_sb[:, :, 0:P], in_=wkt_d[0])
                nc.sync.dma_start(out=kt_in[:, :, 0:512], in_=kt_d[0])
                for fo in range(1, FO):
                    nc.sync.dma_start(
                        out=wk_sb[:, :, fo * P:(fo + 1) * P], in_=wkt_d[fo])
                nc.sync.dma_start(out=kt_in[:, :, 512:1024], in_=kt_d[1])
                wv_sb = io.tile([P, EO, E], BF16)
                vt_all = io.tile([P, KC // 2, EO, P], BF16)
                for fb in range(2):
                    nc.sync.dma_start(
                        out=wv_sb[:, :, fb * 512:(fb + 1) * 512], in_=wvt_d[fb])
                for kc in range(KC // 2):
                    nc.sync.dma_start(out=vt_all[:, kc], in_=vt_d[kc])
                wq_sb = io.tile([P, EO, E], BF16)
                qt_in = io.tile([P, EO, SQ], BF16)
                for fo in range(FO):
                    nc.sync.dma_start(
                        out=wq_sb[:, :, fo * P:(fo + 1) * P], in_=wqt_d[fo])
                for qb in range(QB):
                    nc.sync.dma_start(
                        out=qt_in[:, :, qb * 512:(qb + 1) * 512], in_=qt_d[qb])

                # k projection, own 1024-row half, kb-outer so each 512-col
                # group can be staged + exchanged as soon as it completes.
                # Staging lives in kt_proj[:, :, kb*512..]; the readback
                # rewrites the local halves in absolute rank order.
                # cc staging rides the (idle) gpsimd queue so it isn't FIFO'd
                # behind the big input-priming DMAs on the sync queues; the
                # doorbell naturally follows the staging on the same queue.
                for kb in range(SQ // 512):
                    for fo in range(FO):
                        ps = ppsum.tile([P, 512], F32, tag="pp")
                        for eo in range(EO):
                            nc.tensor.matmul(
                                ps[:],
                                wk_sb[:, eo, fo * P:(fo + 1) * P],
                                kt_in[:, eo, kb * 512:(kb + 1) * 512],
                                start=(eo == 0),
                                stop=(eo == EO - 1),
                            )
                        nc.scalar.activation(
                            kt_proj[:, fo, kb * 512:(kb + 1) * 512],
                            ps[:],
                            mybir.ActivationFunctionType.Identity,
                            bias=bk_sb[:, fo:fo + 1],
                        )
                        nc.gpsimd.dma_start(
                            out=cck_in[kb][:, fo, :],
                            in_=kt_proj[:, fo, kb * 512:(kb + 1) * 512])
                    nc.gpsimd.collective_compute(
                        "AllGather",
                        mybir.AluOpType.bypass,
                        replica_groups=REPLICA_GROUPS,
                        ins=[cck_in[kb][:]],
                        outs=[cck_out[kb][:]],
                    )
                # k readbacks on the scalar queue, after all k evictions so
                # their cc-completion waits can't block later ACT work that
                # PE depends on sooner.
                for kb in range(SQ // 512):
                    for r in range(2):
                        nc.scalar.dma_start(
                            out=kt_proj[:, :,
                                        r * SQ + kb * 512:r * SQ + (kb + 1) * 512],
                            in_=cck_out[kb][r])

                # v projection, own half; staged in v_proj[:, :KC//2, :]
                for kc in range(KC // 2):
                    for fb in range(2):
                        ps = ppsum.tile([P, 512], F32, tag="pp")
                        for eo in range(EO):
                            nc.tensor.matmul(
                                ps[:],
                                vt_all[:, kc, eo, :],
                                wv_sb[:, eo, fb * 512:(fb + 1) * 512],
                                start=(eo == 0),
                                stop=(eo == EO - 1),
                            )
                        nc.vector.tensor_copy(
                            out=v_proj[:, kc, fb * 512:(fb + 1) * 512],
                            in_=ps[:],
                        )
                    nc.gpsimd.dma_start(
                        out=ccv_in[:, kc, :], in_=v_proj[:, kc, :])
                nc.gpsimd.collective_compute(
                    "AllGather",
                    mybir.AluOpType.bypass,
                    replica_groups=REPLICA_GROUPS,
                    ins=[ccv_in[:]],
                    outs=[ccv_out[:]],
                )
                # v readbacks on the sync queue (drained of priming by then;
                # only the phase-B output DMAs sit behind them)
                for r in range(2):
                    nc.sync.dma_start(
                        out=v_proj[:, r * (KC // 2):(r + 1) * (KC // 2), :],
                        in_=ccv_out[r])

                # q projection: psum[f128, q512] = sum_eo WqT[e,f].T @ qT[e,q]
                for fo in range(FO):
                    for qb in range(QB):
                        ps = ppsum.tile([P, 512], F32, tag="pp")
                        for eo in range(EO):
                            nc.tensor.matmul(
                                ps[:],
                                wq_sb[:, eo, fo * P:(fo + 1) * P],
                                qt_in[:, eo, qb * 512:(qb + 1) * 512],
                                start=(eo == 0),
                                stop=(eo == EO - 1),
                            )
                        nc.scalar.activation(
                            qt_proj[:, fo, qb * 512:(qb + 1) * 512],
                            ps[:],
                            mybir.ActivationFunctionType.Identity,
                            bias=bq_sb[:, fo:fo + 1],
                        )

            # ---------------- phase B: attention ----------------
            with (
                tc.tile_pool(name="phb", bufs=2) as phb,
                tc.tile_pool(name="outp", bufs=3) as outp,
            ):
                if with_mask:
                    mask_sb = phb.tile([P, KC, SQ], BF16, tag="mask", bufs=1)
                    nc.sync.dma_start(out=mask_sb[:], in_=mask_d[:])

                # kc slots in collective-readback readiness order (the first
                # kb-half AllGather delivers slots {0-3, 8-11})
                KC_ORDER = [0, 1, 2, 3, 8, 9, 10, 11, 4, 5, 6, 7, 12, 13, 14, 15]
                for qb in range(QB):
                    # scores + exp for this q 512-block
                    expT = phb.tile([P, KC, 512], BF16, tag="expT")
                    for kc in KC_ORDER:
                        ps = ppsum.tile([P, 512], F32, tag="pp")
                        for fo in range(FO):
                            nc.tensor.matmul(
                                ps[:],
                                kt_proj[:, fo, kc * P:(kc + 1) * P],
                                qt_proj[:, fo, qb * 512:(qb + 1) * 512],
                                start=(fo == 0),
                                stop=(fo == FO - 1),
                            )
                        if with_mask:
                            nc.vector.tensor_scalar_mul(ps[:], ps[:], INV_SCALE)
                            nc.vector.tensor_add(
                                ps[:], ps[:],
                                mask_sb[:, kc, qb * 512:(qb + 1) * 512],
                            )
                            nc.scalar.activation(
                                expT[:, kc, :], ps[:],
                                mybir.ActivationFunctionType.Exp,
                            )
                        else:
                            nc.scalar.activation(
                                expT[:, kc, :], ps[:],
                                mybir.ActivationFunctionType.Exp,
                                scale=INV_SCALE,
                            )

                    # attn @ V (+ row sums) per 128-row q tile
                    for qi in range(4):
                        qg = qb * 4 + qi
                        pa = opsum_a.tile([P, 512], F32, tag="pa")
                        pb = opsum_b.tile([P, 512], F32, tag="pb")
                        psum = spsum.tile([P, 1], F32, tag="psums")
                        for kc in range(KC):
                            lhsT = expT[:, kc, qi * P:(qi + 1) * P]
                            st, sp = (kc == 0), (kc == KC - 1)
                            nc.tensor.matmul(pa[:], lhsT, v_proj[:, kc, 0:512],
                                             start=st, stop=sp)
                            nc.tensor.matmul(pb[:], lhsT, v_proj[:, kc, 512:1024],
                                             start=st, stop=sp)
                            nc.tensor.matmul(psum[:], lhsT, ones_sb[:],
                                             start=st, stop=sp)
                        recip = outp.tile([P, 1], F32, tag="recip")
                        nc.vector.reciprocal(recip[:], psum[:])
                        out_sb = outp.tile([P, E], F32, tag="outsb")
                        nc.scalar.activation(
                            out_sb[:, 0:512], pa[:],
                            mybir.ActivationFunctionType.Copy,
                            scale=recip[:],
                        )
                        nc.scalar.activation(
                            out_sb[:, 512:1024], pb[:],
                            mybir.ActivationFunctionType.Copy,
                            scale=recip[:],
                        )
                        nc.sync.dma_start(out=out_d[:, qg, :], in_=out_sb[:])

    nc.compile()
    return nc


def _bf16_tiled(x):
    """[R, C] fp32 -> [128, R//128, C] bf16 with partition = inner row index."""
    r, c = x.shape
    return (
        np.ascontiguousarray(x).astype(ml_dtypes.bfloat16)
        .reshape(r // P, P, c).transpose(1, 0, 2).copy()
    )


def _prepare_in_maps(query, key, value, attn_mask, Wq, bq, Wk, bk, Wv, bv,
                     with_mask):
    query = np.asarray(query, np.float32)
    key = np.asarray(key, np.float32)
    value = np.asarray(value, np.float32)
    w_t = {}
    for name, w in (("wqt", Wq), ("wkt", Wk)):
        # [fo, p(e-inner), eo, 128(f)] slabs
        w_t[name] = (
            _bf16_tiled(np.asarray(w, np.float32).T)
            .reshape(P, EO, FO, P).transpose(2, 0, 1, 3).copy()
        )
    # wvt: [fb, p(e-inner), eo, 512(f)] slabs
    w_t["wvt"] = (
        _bf16_tiled(np.asarray(Wv, np.float32).T)
        .reshape(P, EO, 2, 512).transpose(2, 0, 1, 3).copy()
    )
    bq_t = np.asarray(bq, np.float32).reshape(FO, P).T.copy()
    bk_t = np.asarray(bk, np.float32).reshape(FO, P).T.copy()

    in_maps = []
    for c in range(N_CORES):
        b, h = c // 2, c % 2
        # qt: [qb, p(e-inner), eo, 512(q)] slabs
        qt = (
            _bf16_tiled(query[b, h * SQ:(h + 1) * SQ, :].T)
            .reshape(P, EO, QB, 512).transpose(2, 0, 1, 3).copy()
        )
        # kt: [kb, p(e-inner), eo, 512(k)] slabs of the own half
        kt = (
            _bf16_tiled(key[b].T[:, h * SQ:(h + 1) * SQ])
            .reshape(P, EO, 2, 512).transpose(2, 0, 1, 3).copy()
        )
        # vt slabs for own k-half: [kc_local, p(e-inner), eo, w(k-inner)]
        vt = (
            np.ascontiguousarray(value[b].T[:, h * SQ:(h + 1) * SQ])
            .astype(ml_dtypes.bfloat16)
            .reshape(EO, P, KC // 2, P).transpose(2, 1, 0, 3).copy()
        )
        m = dict(qt=qt, kt=kt, vt=vt, bq=bq_t, bk=bk_t, **w_t)
        if with_mask:
            mt = np.asarray(attn_mask[b, h * SQ:(h + 1) * SQ, :], np.float32).T
            m["maskt"] = (
                mt.astype(ml_dtypes.bfloat16)
                .reshape(KC, P, SQ).transpose(1, 0, 2).copy()
            )
        in_maps.append(m)
    return in_maps


def _run(inputs, trace=False):
    with_mask = bool(np.any(np.asarray(inputs["attn_mask"])))
    key = with_mask
    if key not in _BUILD_CACHE:
        _BUILD_CACHE[key] = _build(with_mask)
    nc = _BUILD_CACHE[key]

    in_maps = _prepare_in_maps(with_mask=with_mask, **inputs)
    res = run_bass_kernel_spmd(nc, in_maps, core_ids=list(range(N_CORES)),
                               trace=trace)

    bv = np.asarray(inputs["bv"], np.float32)
    out = np.zeros((B, S, E), np.float32)
    for c in range(N_CORES):
        b, h = c // 2, c % 2
        oc = res.results[c]["out"]  # [P, SQ//P, E]
        out[b, h * SQ:(h + 1) * SQ, :] = (
            oc.transpose(1, 0, 2).reshape(SQ, E) + bv[None, :]
        )
    return out, res


def kernel(**inputs) -> np.ndarray:
    out, _ = _run(inputs, trace=False)
    return out


# revision 20
# speedup vs baseline: 1.1699x; 1.1699x over previous
"""Self-contained Trainium2 Bass kernel for nn_AttentionModel (B=4, S=2048, E=1024).

Model: q/k/v linear projections + scaled-dot-product attention (scale = sqrt(E)).

Sharding (8 NeuronCores): core c handles batch b=c//2, query-row half h=c%2
(1024 q rows). k/v projections are split across the core pair (each projects
its own 1024 k-rows) and exchanged with a pair-local AllGather
(replica_groups [[0,1],[2,3],[4,5],[6,7]]); both halves are read back from
the collective output in absolute rank order, so the program stays fully
SPMD-uniform with no core-dependent addressing.

Device algorithm per core (all matmuls bf16 with fp32 PSUM accumulation):
  qT_proj[f,q]  = WqT.T @ qT_in   (+bq via ACT bias on eviction)
  kT_proj[f,k]  = WkT.T @ kT_in   (+bk)   [own half, then pair AllGather]
  v_proj [k,f]  = vT_in.T @ WvT   [own half, then pair AllGather;
                                   bias bv applied on host after gather]
  scoresT[k,q]  = kT_proj.T @ qT_proj          (per 128k x 512q psum tile)
  expT   [k,q]  = exp(scoresT / sqrt(E))       (ACT, no max-subtraction:
                                                logits are O(+-6), fp32-safe)
  out_un [q,f]  = expT.T @ v_proj              (accumulate over k chunks)
  sums   [q,1]  = expT.T @ ones                (same stationary operand)
  out    [q,f]  = out_un * (1/sums)            (per-partition ACT scale)

Host pre-tiles every input into the [128, outer, free] SBUF layout in bf16, so
the device performs no transposes or casts on the inputs.
"""

import sys

for _p in ("/opt/trn_rl_repo", "/root/.axon_site/_ro/trn_rl_repo"):
    if _p not in sys.path:
        sys.path.insert(0, _p)

import numpy as np
import ml_dtypes

import concourse.bacc as bacc
import concourse.mybir as mybir
import concourse.tile as tile
from concourse.bass_utils import run_bass_kernel_spmd

B, S, E = 4, 2048, 1024
P = 128
SQ = S // 2          # q rows per core
N_CORES = 8
EO = E // P          # 8  e-outer chunks
FO = E // P          # 8  f-outer chunks
KC = S // P          # 16 k-row chunks
QB = SQ // 512       # 2  q 512-blocks
INV_SCALE = float(1.0 / np.sqrt(np.float32(E)))

BF16 = mybir.dt.bfloat16
F32 = mybir.dt.float32

_BUILD_CACHE: dict = {}


def _build(with_mask: bool):
    nc = bacc.Bacc(
        "TRN2",
        target_bir_lowering=False,
        debug=False,
        enable_asserts=False,
        num_devices=N_CORES,
    )

    # qt/kt/wqt/wkt arrive as fo/qb/kb slabs so the first matmul chain only
    # needs ~0.75MB of DMA before PE can start.
    qt_d = nc.declare_dram_parameter("qt", [QB, P, EO, 512], BF16, isOutput=False)
    kt_d = nc.declare_dram_parameter("kt", [2, P, EO, 512], BF16, isOutput=False)
    vt_d = nc.declare_dram_parameter("vt", [KC // 2, P, EO, P], BF16, isOutput=False)
    wqt_d = nc.declare_dram_parameter("wqt", [FO, P, EO, P], BF16, isOutput=False)
    wkt_d = nc.declare_dram_parameter("wkt", [FO, P, EO, P], BF16, isOutput=False)
    wvt_d = nc.declare_dram_parameter("wvt", [2, P, EO, 512], BF16, isOutput=False)
    bq_d = nc.declare_dram_parameter("bq", [P, FO], F32, isOutput=False)
    bk_d = nc.declare_dram_parameter("bk", [P, FO], F32, isOutput=False)
    if with_mask:
        mask_d = nc.declare_dram_parameter("maskt", [P, KC, SQ], BF16, isOutput=False)
    out_d = nc.declare_dram_parameter("out", [P, SQ // P, E], F32, isOutput=True)

    # pair-AllGather staging buffers (internal DRAM)
    # NOTE: Shared-output collectives need >4-core groups; for 2-core pair
    # groups the output must stay a Local internal tensor. The k exchange is
    # split into two kb-halves so the first AllGather launches ~15us earlier
    # and scores can start on its chunks while the rest is in flight.
    cck_in = [nc.dram_tensor(f"cck_in{kb}", [P, FO, 512], BF16) for kb in range(2)]
    cck_out = [nc.dram_tensor(f"cck_out{kb}", [2, P, FO, 512], BF16)
               for kb in range(2)]
    ccv_in = nc.dram_tensor("ccv_in", [P, KC // 2, E], BF16)
    ccv_out = nc.dram_tensor("ccv_out", [2, P, KC // 2, E], BF16)
    REPLICA_GROUPS = [[0, 1], [2, 3], [4, 5], [6, 7]]

    with tile.TileContext(nc) as tc:
        with (
            tc.tile_pool(name="const", bufs=1) as const,
            tc.tile_pool(name="proj", bufs=1) as proj,
            tc.tile_pool(name="ppsum", bufs=2, space="PSUM") as ppsum,
            tc.tile_pool(name="opsum_a", bufs=2, space="PSUM") as opsum_a,
            tc.tile_pool(name="opsum_b", bufs=2, space="PSUM") as opsum_b,
            tc.tile_pool(name="spsum", bufs=2, space="PSUM") as spsum,
        ):
            ones_sb = const.tile([P, 1], BF16)
            nc.any.memset(ones_sb[:], 1.0)
            bq_sb = const.tile([P, FO], F32)
            nc.sync.dma_start(out=bq_sb[:], in_=bq_d[:])
            bk_sb = const.tile([P, FO], F32)
            nc.sync.dma_start(out=bk_sb[:], in_=bk_d[:])

            # persistent projected tensors
            qt_proj = proj.tile([P, FO, SQ], BF16)   # [f-inner, fo, q]
            kt_proj = proj.tile([P, FO, S], BF16)    # [f-inner, fo, k]
            v_proj = proj.tile([P, KC, E], BF16)     # [k-inner, kc, f]

            # ---------------- phase A: projections ----------------
            with tc.tile_pool(name="io", bufs=1) as io:
                # Priming DMAs in consumption order: k-proj inputs first (its
                # AllGathers must launch earliest), then v, then q.
                wk_sb = io.tile([P, EO, E], BF16)
                kt_in = io.tile([P, EO, SQ], BF16)
                nc.sync.dma_start(out=wk_sb[:, :, 0:P], in_=wkt_d[0])
                nc.sync.dma_start(out=kt_in[:, :, 0:512], in_=kt_d[0])
                for fo in range(1, FO):
                    nc.sync.dma_start(
                        out=wk_sb[:, :, fo * P:(fo + 1) * P], in_=wkt_d[fo])
                nc.sync.dma_start(out=kt_in[:, :, 512:1024], in_=kt_d[1])
                wv_sb = io.tile([P, EO, E], BF16)
                vt_all = io.tile([P, KC // 2, EO, P], BF16)
                for fb in range(2):
                    nc.sync.dma_start(
                        out=wv_sb[:, :, fb * 512:(fb + 1) * 512], in_=wvt_d[fb])
                for kc in range(KC // 2):
                    nc.sync.dma_start(out=vt_all[:, kc], in_=vt_d[kc])
                wq_sb = io.tile([P, EO, E], BF16)
                qt_in = io.tile([P, EO, SQ], BF16)
                for fo in range(FO):
                    nc.sync.dma_start(
                        out=wq_sb[:, :, fo * P:(fo + 1) * P], in_=wqt_d[fo])
                for qb in range(QB):
                    nc.sync.dma_start(
                        out=qt_in[:, :, qb * 512:(qb + 1) * 512], in_=qt_d[qb])

                # k projection, own 1024-row half, kb-outer so each 512-col
                # group can be staged + exchanged as soon as it completes.
                # Staging lives in kt_proj[:, :, kb*512..]; the readback
                # rewrites the local halves in absolute rank order.
                # cc staging rides the (idle) gpsimd queue so it isn't FIFO'd
                # behind the big input-priming DMAs on the sync queues; the
                # doorbell naturally follows the staging on the same queue.
                for kb in range(SQ // 512):
                    for fo in range(FO):
                        ps = ppsum.tile([P, 512], F32, tag="pp")
                        for eo in range(EO):
                            nc.tensor.matmul(
                                ps[:],
                                wk_sb[:, eo, fo * P:(fo + 1) * P],
                                kt_in[:, eo, kb * 512:(kb + 1) * 512],
                                start=(eo == 0),
                                stop=(eo == EO - 1),
                            )
                        nc.scalar.activation(
                            kt_proj[:, fo, kb * 512:(kb + 1) * 512],
                            ps[:],
                            mybir.ActivationFunctionType.Identity,
                            bias=bk_sb[:, fo:fo + 1],
                        )
                        nc.gpsimd.dma_start(
                            out=cck_in[kb][:, fo, :],
                            in_=kt_proj[:, fo, kb * 512:(kb + 1) * 512])
                    nc.gpsimd.collective_compute(
                        "AllGather",
                        mybir.AluOpType.bypass,
                        replica_groups=REPLICA_GROUPS,
                        ins=[cck_in[kb][:]],
                        outs=[cck_out[kb][:]],
                    )
                # k readbacks on the sync queue (priming has drained by the
                # time the first AllGather completes; nothing later on this
                # queue is needed before the readbacks).
                for kb in range(SQ // 512):
                    for r in range(2):
                        nc.sync.dma_start(
                            out=kt_proj[:, :,
                                        r * SQ + kb * 512:r * SQ + (kb + 1) * 512],
                            in_=cck_out[kb][r])

                # v projection, own half; staged in v_proj[:, :KC//2, :]
                for kc in range(KC // 2):
                    for fb in range(2):
                        ps = ppsum.tile([P, 512], F32, tag="pp")
                        for eo in range(EO):
                            nc.tensor.matmul(
                                ps[:],
                                vt_all[:, kc, eo, :],
                                wv_sb[:, eo, fb * 512:(fb + 1) * 512],
                                start=(eo == 0),
                                stop=(eo == EO - 1),
                            )
                        nc.vector.tensor_copy(
                            out=v_proj[:, kc, fb * 512:(fb + 1) * 512],
                            in_=ps[:],
                        )
                    nc.gpsimd.dma_start(
                        out=ccv_in[:, kc, :], in_=v_proj[:, kc, :])
                nc.gpsimd.collective_compute(
                    "AllGather",
                    mybir.AluOpType.bypass,
                    replica_groups=REPLICA_GROUPS,
                    ins=[ccv_in[:]],
                    outs=[ccv_out[:]],
                )
                # v readbacks on the sync queue (drained of priming by then;
                # only the phase-B output DMAs sit behind them)
                for r in range(2):
                    nc.sync.dma_start(
                        out=v_proj[:, r * (KC // 2):(r + 1) * (KC // 2), :],
                        in_=ccv_out[r])

                # q projection: psum[f128, q512] = sum_eo WqT[e,f].T @ qT[e,q]
                for fo in range(FO):
                    for qb in range(QB):
                        ps = ppsum.tile([P, 512], F32, tag="pp")
                        for eo in range(EO):
                            nc.tensor.matmul(
                                ps[:],
                                wq_sb[:, eo, fo * P:(fo + 1) * P],
                                qt_in[:, eo, qb * 512:(qb + 1) * 512],
                                start=(eo == 0),
                                stop=(eo == EO - 1),
                            )
                        nc.scalar.activation(
                            qt_proj[:, fo, qb * 512:(qb + 1) * 512],
                            ps[:],
                            mybir.ActivationFunctionType.Identity,
                            bias=bq_sb[:, fo:fo + 1],
                        )

            # ---------------- phase B: attention ----------------
            with (
                tc.tile_pool(name="phb", bufs=2) as phb,
                tc.tile_pool(name="outp", bufs=3) as outp,
            ):
                if with_mask:
                    mask_sb = phb.tile([P, KC, SQ], BF16, tag="mask", bufs=1)
                    nc.sync.dma_start(out=mask_sb[:], in_=mask_d[:])

                # kc slots in collective-readback readiness order (the first
                # kb-half AllGather delivers slots {0-3, 8-11})
                KC_ORDER = [0, 1, 2, 3, 8, 9, 10, 11, 4, 5, 6, 7, 12, 13, 14, 15]
                for qb in range(QB):
                    # scores + exp for this q 512-block
                    expT = phb.tile([P, KC, 512], BF16, tag="expT")
                    for kc in KC_ORDER:
                        ps = ppsum.tile([P, 512], F32, tag="pp")
                        for fo in range(FO):
                            nc.tensor.matmul(
                                ps[:],
                                kt_proj[:, fo, kc * P:(kc + 1) * P],
                                qt_proj[:, fo, qb * 512:(qb + 1) * 512],
                                start=(fo == 0),
                                stop=(fo == FO - 1),
                            )
                        if with_mask:
                            nc.vector.tensor_scalar_mul(ps[:], ps[:], INV_SCALE)
                            nc.vector.tensor_add(
                                ps[:], ps[:],
                                mask_sb[:, kc, qb * 512:(qb + 1) * 512],
                            )
                            nc.scalar.activation(
                                expT[:, kc, :], ps[:],
                                mybir.ActivationFunctionType.Exp,
                            )
                        else:
                            nc.scalar.activation(
                                expT[:, kc, :], ps[:],
                                mybir.ActivationFunctionType.Exp,
                                scale=INV_SCALE,
                            )

                    # attn @ V (+ row sums) per 128-row q tile
                    for qi in range(4):
                        qg = qb * 4 + qi
                        pa = opsum_a.tile([P, 512], F32, tag="pa")
                        pb = opsum_b.tile([P, 512], F32, tag="pb")
                        psum = spsum.tile([P, 1], F32, tag="psums")
                        for kc in range(KC):
                            lhsT = expT[:, kc, qi * P:(qi + 1) * P]
                            st, sp = (kc == 0), (kc == KC - 1)
                            nc.tensor.matmul(pa[:], lhsT, v_proj[:, kc, 0:512],
                                             start=st, stop=sp)
                            nc.tensor.matmul(pb[:], lhsT, v_proj[:, kc, 512:1024],
                                             start=st, stop=sp)
                            nc.tensor.matmul(psum[:], lhsT, ones_sb[:],
                                             start=st, stop=sp)
                        recip = outp.tile([P, 1], F32, tag="recip")
                        nc.vector.reciprocal(recip[:], psum[:])
                        out_sb = outp.tile([P, E], F32, tag="outsb")
                        nc.scalar.activation(
                            out_sb[:, 0:512], pa[:],
                            mybir.ActivationFunctionType.Copy,
                            scale=recip[:],
                        )
                        nc.scalar.activation(
                            out_sb[:, 512:1024], pb[:],
                            mybir.ActivationFunctionType.Copy,
                            scale=recip[:],
                        )
                        nc.sync.dma_start(out=out_d[:, qg, :], in_=out_sb[:])

    nc.compile()
    return nc


def _bf16_tiled(x):
    """[R, C] fp32 -> [128, R//128, C] bf16 with partition = inner row index."""
    r, c = x.shape
    return (
        np.ascontiguousarray(x).astype(ml_dtypes.bfloat16)
        .reshape(r // P, P, c).transpose(1, 0, 2).copy()
    )


def _prepare_in_maps(query, key, value, attn_mask, Wq, bq, Wk, bk, Wv, bv,
                     with_mask):
    query = np.asarray(query, np.float32)
    key = np.asarray(key, np.float32)
    value = np.asarray(value, np.float32)
    w_t = {}
    for name, w in (("wqt", Wq), ("wkt", Wk)):
        # [fo, p(e-inner), eo, 128(f)] slabs
        w_t[name] = (
            _bf16_tiled(np.asarray(w, np.float32).T)
            .reshape(P, EO, FO, P).transpose(2, 0, 1, 3).copy()
        )
    # wvt: [fb, p(e-inner), eo, 512(f)] slabs
    w_t["wvt"] = (
        _bf16_tiled(np.asarray(Wv, np.float32).T)
        .reshape(P, EO, 2, 512).transpose(2, 0, 1, 3).copy()
    )
    bq_t = np.asarray(bq, np.float32).reshape(FO, P).T.copy()
    bk_t = np.asarray(bk, np.float32).reshape(FO, P).T.copy()

    in_maps = []
    for c in range(N_CORES):
        b, h = c // 2, c % 2
        # qt: [qb, p(e-inner), eo, 512(q)] slabs
        qt = (
            _bf16_tiled(query[b, h * SQ:(h + 1) * SQ, :].T)
            .reshape(P, EO, QB, 512).transpose(2, 0, 1, 3).copy()
        )
        # kt: [kb, p(e-inner), eo, 512(k)] slabs of the own half
        kt = (
            _bf16_tiled(key[b].T[:, h * SQ:(h + 1) * SQ])
            .reshape(P, EO, 2, 512).transpose(2, 0, 1, 3).copy()
        )
        # vt slabs for own k-half: [kc_local, p(e-inner), eo, w(k-inner)]
        vt = (
            np.ascontiguousarray(value[b].T[:, h * SQ:(h + 1) * SQ])
            .astype(ml_dtypes.bfloat16)
            .reshape(EO, P, KC // 2, P).transpose(2, 1, 0, 3).copy()
        )
        m = dict(qt=qt, kt=kt, vt=vt, bq=bq_t, bk=bk_t, **w_t)
        if with_mask:
            mt = np.asarray(attn_mask[b, h * SQ:(h + 1) * SQ, :], np.float32).T
            m["maskt"] = (
                mt.astype(ml_dtypes.bfloat16)
                .reshape(KC, P, SQ).transpose(1, 0, 2).copy()
            )
        in_maps.append(m)
    return in_maps


def _run(inputs, trace=False):
    with_mask = bool(np.any(np.asarray(inputs["attn_mask"])))
    key = with_mask
    if key not in _BUILD_CACHE:
        _BUILD_CACHE[key] = _build(with_mask)
    nc = _BUILD_CACHE[key]

    in_maps = _prepare_in_maps(with_mask=with_mask, **inputs)
    res = run_bass_kernel_spmd(nc, in_maps, core_ids=list(range(N_CORES)),
                               trace=trace)

    bv = np.asarray(inputs["bv"], np.float32)
    out = np.zeros((B, S, E), np.float32)
    for c in range(N_CORES):
        b, h = c // 2, c % 2
        oc = res.results[c]["out"]  # [P, SQ//P, E]
        out[b, h * SQ:(h + 1) * SQ, :] = (
            oc.transpose(1, 0, 2).reshape(SQ, E) + bv[None, :]
        )
    return out, res


def kernel(**inputs) -> np.ndarray:
    out, _ = _run(inputs, trace=False)
    return out
